# revision 1
# baseline (speedup 1.0000x reference)
"""Dcls1d (dilated conv with learnable spacings, depthwise) Trainium2 kernel.

Problem: x [16, 256, 8192] f32, depthwise conv per channel with a 56-wide
kernel holding 7 interpolated taps (positions = k*8+4 + P, linear interp),
padding 27/27, plus bias.  Output [16, 256, 8191] f32.

Strategy:
  - Data-parallel over batch: 2 images per NeuronCore (8 cores).
  - Host precomputes, per channel c and tap k: integer base shift i0[c,k]
    and the two interpolation coefficients a[c,k] (weight*(1-r)) and
    b[c,k] (weight*r) reading x_pad at offsets i0 and i0+1.
  - Per core the padded input lives in DRAM as 512 rows (2 batches x 256
    channels) of width 8256: [27 zeros][8192 data][37 zeros] so every
    shifted read is in-bounds and the zeros realize the conv padding.
  - For each (channel-group, chunk) tile the kernel issues 7 indirect
    DMA gathers (per-partition row offsets = row*8256 + i0[c,k]) into
    SBUF, then runs 14 scalar_tensor_tensor multiply-accumulate passes
    on the Vector engine (per-partition scalar coefficients), and one
    tensor_scalar pass folding the bias.
"""

import os
from contextlib import ExitStack

import numpy as np

import concourse.bass as bass
import concourse.bacc as bacc_mod
import concourse.mybir as mybir
import concourse.tile as tile
from concourse.bass_utils import run_bass_kernel_spmd

# Problem geometry (hardcoded per spec nn_Dcls1d_12713103196284)
N, C, L = 16, 256, 8192
OUT_L = 8191
KS, DIL, PAD = 7, 8, 27
LK = DIL * KS  # 56
N_CORES = 8
NB = N // N_CORES  # batches per core
ROWS = NB * C  # 512 rows per core
PADW = 8256  # [27 zeros][8192][37 zeros], 64B-aligned rows
CHUNK = 2048
CHUNKS = [(0, 2048), (2048, 2048), (4096, 2048), (6144, 2047)]
GROUPS_PER_C = C // 128  # 2
NTILES = NB * GROUPS_PER_C  # 4

F32 = mybir.dt.float32
F16 = mybir.dt.float16
I32 = mybir.dt.int32
SUB = 512  # matmul subchunk (one PSUM bank of f32)

_PROG = None
_PROG_IMPL = None
LAST_RESULTS = None  # test harness reads exec_time_ns off this


def _build_program_pe():
    """TensorE variant: fp16 gathers; per (tap, a/b) a diagonal 128x128 fp16
    lhsT scales the shifted slice per-channel and accumulates into PSUM
    (fp32); ScalarE evacuates PSUM with the bias add; one DMA store per
    2048-chunk."""
    nc = bacc_mod.Bacc()
    xpad = nc.dram_tensor("xpad", [ROWS, PADW], F16, kind="ExternalInput")
    idx = nc.dram_tensor("idx", [128, NTILES * KS], I32, kind="ExternalInput")
    diags = nc.dram_tensor(
        "diags", [128, GROUPS_PER_C * KS * 2 * 128], F16, kind="ExternalInput"
    )
    cbias = nc.dram_tensor("cbias", [128, GROUPS_PER_C], F32, kind="ExternalInput")
    out = nc.dram_tensor("out", [ROWS, OUT_L], F32, kind="ExternalOutput")

    with ExitStack() as ctx:
        tc = ctx.enter_context(tile.TileContext(nc))
        const = ctx.enter_context(tc.tile_pool(name="const", bufs=1))
        idx_sb = const.tile([128, NTILES * KS], I32)
        nc.sync.dma_start(idx_sb[:], idx[:])
        diag_sb = const.tile([128, GROUPS_PER_C * KS * 2 * 128], F16)
        nc.sync.dma_start(diag_sb[:], diags[:])
        cbias_sb = const.tile([128, GROUPS_PER_C], F32)
        nc.sync.dma_start(cbias_sb[:], cbias[:])

        xs_pool = ctx.enter_context(tc.tile_pool(name="xs", bufs=2))
        psum_pool = ctx.enter_context(
            tc.tile_pool(name="ps", bufs=8, space="PSUM")
        )
        ev_pool = ctx.enter_context(tc.tile_pool(name="ev", bufs=2))

        for t in range(NTILES):
            b, g = divmod(t, GROUPS_PER_C)
            row0 = b * C + g * 128
            for c0, w in CHUNKS:
                xs = [
                    xs_pool.tile([128, CHUNK + 1], F16, tag=f"xs{k}", name=f"xs{k}")
                    for k in range(KS)
                ]
                for k in range(KS):
                    col = t * KS + k
                    nc.gpsimd.indirect_dma_start(
                        out=xs[k][:, 0 : w + 1],
                        out_offset=None,
                        in_=xpad[:],
                        in_offset=bass.IndirectOffsetOnAxis(
                            ap=idx_sb[:, col : col + 1], axis=1
                        ),
                        element_offset=c0,
                    )
                ev = ev_pool.tile([128, CHUNK], F32)
                for s in range(CHUNK // SUB):
                    s0 = s * SUB
                    sw = min(SUB, w - s0)
                    ps = psum_pool.tile([128, SUB], F32)
                    for k in range(KS):
                        j = (g * KS + k) * 2
                        nc.tensor.matmul(
                            out=ps[:, 0:sw],
                            lhsT=diag_sb[:, j * 128 : (j + 1) * 128],
                            rhs=xs[k][:, s0 : s0 + sw],
                            start=(k == 0),
                            stop=False,
                        )
                        nc.tensor.matmul(
                            out=ps[:, 0:sw],
                            lhsT=diag_sb[:, (j + 1) * 128 : (j + 2) * 128],
                            rhs=xs[k][:, s0 + 1 : s0 + 1 + sw],
                            start=False,
                            stop=(k == KS - 1),
                        )
                    nc.scalar.activation(
                        ev[:, s0 : s0 + sw],
                        ps[:, 0:sw],
                        mybir.ActivationFunctionType.Identity,
                        bias=cbias_sb[:, g : g + 1],
                        scale=1.0,
                    )
                nc.sync.dma_start(out[row0 : row0 + 128, c0 : c0 + w], ev[:, 0:w])
    nc.finalize()
    return nc


CHUNK2 = 4096
CHUNKS2 = [(0, 4096), (4096, 4095)]


def _build_program_pe2():
    """Like _build_program_pe, but: fp16 output stores, 4096-wide chunks,
    and every third 512-subchunk computed on the (otherwise idle) Vector
    engine via fp16 scalar_tensor_tensor chains to relieve both the DMA
    (smaller stores) and TensorE (fewer matmuls)."""
    nc = bacc_mod.Bacc()
    xpad = nc.dram_tensor("xpad", [ROWS, PADW], F16, kind="ExternalInput")
    idx = nc.dram_tensor("idx", [128, NTILES * KS], I32, kind="ExternalInput")
    diags = nc.dram_tensor(
        "diags", [128, GROUPS_PER_C * KS * 2 * 128], F16, kind="ExternalInput"
    )
    ca = nc.dram_tensor("ca", [128, GROUPS_PER_C * KS], F32, kind="ExternalInput")
    cb = nc.dram_tensor("cb", [128, GROUPS_PER_C * KS], F32, kind="ExternalInput")
    cbias = nc.dram_tensor("cbias", [128, GROUPS_PER_C], F32, kind="ExternalInput")
    out = nc.dram_tensor("out", [ROWS, OUT_L], F16, kind="ExternalOutput")

    mult = mybir.AluOpType.mult
    add = mybir.AluOpType.add

    with ExitStack() as ctx:
        tc = ctx.enter_context(tile.TileContext(nc))
        const = ctx.enter_context(tc.tile_pool(name="const", bufs=1))
        idx_sb = const.tile([128, NTILES * KS], I32)
        nc.sync.dma_start(idx_sb[:], idx[:])
        diag_sb = const.tile([128, GROUPS_PER_C * KS * 2 * 128], F16)
        nc.sync.dma_start(diag_sb[:], diags[:])
        ca_sb = const.tile([128, GROUPS_PER_C * KS], F32)
        nc.sync.dma_start(ca_sb[:], ca[:])
        cb_sb = const.tile([128, GROUPS_PER_C * KS], F32)
        nc.sync.dma_start(cb_sb[:], cb[:])
        cbias_sb = const.tile([128, GROUPS_PER_C], F32)
        nc.sync.dma_start(cbias_sb[:], cbias[:])

        xs_pool = ctx.enter_context(tc.tile_pool(name="xs", bufs=3))
        psum_pool = ctx.enter_context(tc.tile_pool(name="ps", bufs=6, space="PSUM"))
        psd_pool = ctx.enter_context(tc.tile_pool(name="psd", bufs=1, space="PSUM"))
        ev_pool = ctx.enter_context(tc.tile_pool(name="ev", bufs=3))

        for t in range(NTILES):
            b, g = divmod(t, GROUPS_PER_C)
            row0 = b * C + g * 128
            for c0, w in CHUNKS2:
                xs = [
                    xs_pool.tile(
                        [128, CHUNK2 + 1], F16, tag=f"xs{k}", name=f"xs{k}"
                    )
                    for k in range(KS)
                ]
                for k in range(KS):
                    col = t * KS + k
                    nc.gpsimd.indirect_dma_start(
                        out=xs[k][:, 0 : w + 1],
                        out_offset=None,
                        in_=xpad[:],
                        in_offset=bass.IndirectOffsetOnAxis(
                            ap=idx_sb[:, col : col + 1], axis=1
                        ),
                        element_offset=c0,
                    )
                ev = ev_pool.tile([128, CHUNK2], F16)
                cc = g * KS
                nsub = (w + SUB - 1) // SUB
                pe_subs = (nsub * 3) // 4  # leading 3/4 on PE, tail on DVE
                for s in range(pe_subs + 1):
                    is_dve = s == pe_subs
                    s0 = s * SUB
                    sw = min(SUB, w - s0) if not is_dve else w - s0
                    evs = ev[:, s0 : s0 + sw]
                    if is_dve:
                        # VectorE subchunk: multiply-accumulate chain into a
                        # fp32 PSUM accumulator (STT runs 1x regardless, so
                        # the fp32 accumulator costs nothing extra and keeps
                        # PE-path precision); ScalarE evacuates.
                        pd = psd_pool.tile([128, 2 * SUB], F32, name="pd", tag="psd")
                        pda = pd[:, 0:sw]
                        nc.vector.tensor_scalar(
                            pda,
                            xs[0][:, s0 : s0 + sw],
                            ca_sb[:, cc : cc + 1],
                            cbias_sb[:, g : g + 1],
                            mult,
                            add,
                        )
                        nc.vector.scalar_tensor_tensor(
                            pda,
                            xs[0][:, s0 + 1 : s0 + 1 + sw],
                            cb_sb[:, cc : cc + 1],
                            pda,
                            mult,
                            add,
                        )
                        for k in range(1, KS):
                            nc.vector.scalar_tensor_tensor(
                                pda,
                                xs[k][:, s0 : s0 + sw],
                                ca_sb[:, cc + k : cc + k + 1],
                                pda,
                                mult,
                                add,
                            )
                            nc.vector.scalar_tensor_tensor(
                                pda,
                                xs[k][:, s0 + 1 : s0 + 1 + sw],
                                cb_sb[:, cc + k : cc + k + 1],
                                pda,
                                mult,
                                add,
                            )
                        nc.scalar.activation(
                            evs,
                            pda,
                            mybir.ActivationFunctionType.Copy,
                        )
                    else:
                        ps = psum_pool.tile([128, SUB], F32)
                        for k in range(KS):
                            j = (g * KS + k) * 2
                            nc.tensor.matmul(
                                out=ps[:, 0:sw],
                                lhsT=diag_sb[:, j * 128 : (j + 1) * 128],
                                rhs=xs[k][:, s0 : s0 + sw],
                                start=(k == 0),
                                stop=False,
                            )
                            nc.tensor.matmul(
                                out=ps[:, 0:sw],
                                lhsT=diag_sb[:, (j + 1) * 128 : (j + 2) * 128],
                                rhs=xs[k][:, s0 + 1 : s0 + 1 + sw],
                                start=False,
                                stop=(k == KS - 1),
                            )
                        nc.scalar.activation(
                            evs,
                            ps[:, 0:sw],
                            mybir.ActivationFunctionType.Identity,
                            bias=cbias_sb[:, g : g + 1],
                            scale=1.0,
                        )
                # split store: PE/ACT columns go out while the DVE block
                # still computes its tail columns
                ds = pe_subs * SUB
                nc.sync.dma_start(out[row0 : row0 + 128, c0 : c0 + ds], ev[:, 0:ds])
                nc.sync.dma_start(
                    out[row0 : row0 + 128, c0 + ds : c0 + w], ev[:, ds:w]
                )
    nc.finalize()
    return nc


def _build_program():
    nc = bacc_mod.Bacc()
    xpad = nc.dram_tensor("xpad", [ROWS, PADW], F32, kind="ExternalInput")
    idx = nc.dram_tensor("idx", [128, NTILES * KS], I32, kind="ExternalInput")
    ca = nc.dram_tensor("ca", [128, GROUPS_PER_C * KS], F32, kind="ExternalInput")
    cb = nc.dram_tensor("cb", [128, GROUPS_PER_C * KS], F32, kind="ExternalInput")
    cbias = nc.dram_tensor("cbias", [128, GROUPS_PER_C], F32, kind="ExternalInput")
    out = nc.dram_tensor("out", [ROWS, OUT_L], F32, kind="ExternalOutput")

    mult = mybir.AluOpType.mult
    add = mybir.AluOpType.add

    with ExitStack() as ctx:
        tc = ctx.enter_context(tile.TileContext(nc))
        const = ctx.enter_context(tc.tile_pool(name="const", bufs=1))
        idx_sb = const.tile([128, NTILES * KS], I32)
        nc.sync.dma_start(idx_sb[:], idx[:])
        ca_sb = const.tile([128, GROUPS_PER_C * KS], F32)
        nc.sync.dma_start(ca_sb[:], ca[:])
        cb_sb = const.tile([128, GROUPS_PER_C * KS], F32)
        nc.sync.dma_start(cb_sb[:], cb[:])
        cbias_sb = const.tile([128, GROUPS_PER_C], F32)
        nc.sync.dma_start(cbias_sb[:], cbias[:])

        xs_pool = ctx.enter_context(tc.tile_pool(name="xs", bufs=2))
        acc_pool = ctx.enter_context(tc.tile_pool(name="acc", bufs=3))

        for t in range(NTILES):
            b, g = divmod(t, GROUPS_PER_C)
            row0 = b * C + g * 128
            for c0, w in CHUNKS:
                xs = [
                    xs_pool.tile([128, CHUNK + 1], F32, tag=f"xs{k}", name=f"xs{k}")
                    for k in range(KS)
                ]
                for k in range(KS):
                    col = t * KS + k
                    nc.gpsimd.indirect_dma_start(
                        out=xs[k][:, 0 : w + 1],
                        out_offset=None,
                        in_=xpad[:],
                        in_offset=bass.IndirectOffsetOnAxis(
                            ap=idx_sb[:, col : col + 1], axis=1
                        ),
                        element_offset=c0,
                    )
                acc = acc_pool.tile([128, CHUNK], F32)
                cc = g * KS
                # acc = xs0 * a0 + bias
                nc.vector.tensor_scalar(
                    acc[:, 0:w],
                    xs[0][:, 0:w],
                    ca_sb[:, cc : cc + 1],
                    cbias_sb[:, g : g + 1],
                    mult,
                    add,
                )
                nc.vector.scalar_tensor_tensor(
                    acc[:, 0:w],
                    xs[0][:, 1 : w + 1],
                    cb_sb[:, cc : cc + 1],
                    acc[:, 0:w],
                    mult,
                    add,
                )
                for k in range(1, KS):
                    nc.vector.scalar_tensor_tensor(
                        acc[:, 0:w],
                        xs[k][:, 0:w],
                        ca_sb[:, cc + k : cc + k + 1],
                        acc[:, 0:w],
                        mult,
                        add,
                    )
                    nc.vector.scalar_tensor_tensor(
                        acc[:, 0:w],
                        xs[k][:, 1 : w + 1],
                        cb_sb[:, cc + k : cc + k + 1],
                        acc[:, 0:w],
                        mult,
                        add,
                    )
                nc.sync.dma_start(out[row0 : row0 + 128, c0 : c0 + w], acc[:, 0:w])
    nc.finalize()
    return nc


def _host_taps(weight, P):
    """Mirror reference.construct_kernel's float32 math: per (channel, tap)
    integer shift i0 into the 27-padded row and coefficients a (at i0) and
    b (at i0+1)."""
    w = np.asarray(weight, dtype=np.float32)[:, 0, :]  # [C, KS]
    Pm = np.asarray(P, dtype=np.float32)[0, :, 0, :]  # [C, KS]
    base = (np.arange(KS, dtype=np.float32) * DIL + DIL // 2).astype(np.float32)
    p = np.clip(Pm + base[None, :], np.float32(0.0), np.float32(LK - 1))
    i0f = np.floor(p)
    r = (p - i0f).astype(np.float32)
    i0 = i0f.astype(np.int32)
    i1 = np.minimum(i0 + 1, LK - 1)
    a = (w * (np.float32(1.0) - r)).astype(np.float32)
    bcoef = (w * r).astype(np.float32)
    clipped = i1 == i0  # i0 == 55: both interp points coincide
    a = np.where(clipped, a + bcoef, a)
    bcoef = np.where(clipped, np.float32(0.0), bcoef)
    return i0, a, bcoef


def kernel(x, weight, P, bias):
    global _PROG, _PROG_IMPL, LAST_RESULTS
    impl = os.environ.get("KERNEL_IMPL", "pe2")
    x = np.ascontiguousarray(np.asarray(x, dtype=np.float32))
    bias = np.asarray(bias, dtype=np.float32)
    i0, a, b = _host_taps(weight, P)

    # Per-partition constant tables (identical on every core).
    idx_arr = np.zeros((128, NTILES * KS), dtype=np.int32)
    ca_arr = np.zeros((128, GROUPS_PER_C * KS), dtype=np.float32)
    cb_arr = np.zeros((128, GROUPS_PER_C * KS), dtype=np.float32)
    cbias_arr = np.zeros((128, GROUPS_PER_C), dtype=np.float32)
    for t in range(NTILES):
        bt, g = divmod(t, GROUPS_PER_C)
        row0 = bt * C + g * 128
        ch = g * 128 + np.arange(128)
        for k in range(KS):
            idx_arr[:, t * KS + k] = (row0 + np.arange(128)) * PADW + i0[ch, k]
    for g in range(GROUPS_PER_C):
        ch = g * 128 + np.arange(128)
        for k in range(KS):
            ca_arr[:, g * KS + k] = a[ch, k]
            cb_arr[:, g * KS + k] = b[ch, k]
        cbias_arr[:, g] = bias[ch]

    # Pad per-core shards: rows [27 zeros][8192][37 zeros].
    xr = x.reshape(N_CORES, ROWS, L)
    xdt = np.float16 if impl in ("pe", "pe2") else np.float32
    xpad_all = np.zeros((N_CORES, ROWS, PADW), dtype=xdt)
    xpad_all[:, :, PAD : PAD + L] = xr

    if _PROG is None or _PROG_IMPL != impl:
        builders = {"pe": _build_program_pe, "pe2": _build_program_pe2, "dve": _build_program}
        _PROG = builders[impl]()
        _PROG_IMPL = impl
    nc = _PROG

    if impl in ("pe", "pe2"):
        diag_arr = np.zeros((128, GROUPS_PER_C * KS * 2 * 128), dtype=np.float16)
        rows128 = np.arange(128)
        for g in range(GROUPS_PER_C):
            ch = g * 128 + rows128
            for k in range(KS):
                j = (g * KS + k) * 2
                diag_arr[rows128, j * 128 + rows128] = a[ch, k].astype(np.float16)
                diag_arr[rows128, (j + 1) * 128 + rows128] = b[ch, k].astype(
                    np.float16
                )
        in_maps = [
            {
                "xpad": xpad_all[i],
                "idx": idx_arr,
                "diags": diag_arr,
                "cbias": cbias_arr,
            }
            for i in range(N_CORES)
        ]
        if impl == "pe2":
            for m in in_maps:
                m["ca"] = ca_arr
                m["cb"] = cb_arr
    else:
        in_maps = [
            {
                "xpad": xpad_all[i],
                "idx": idx_arr,
                "ca": ca_arr,
                "cb": cb_arr,
                "cbias": cbias_arr,
            }
            for i in range(N_CORES)
        ]
    trace = bool(int(os.environ.get("KERNEL_TRACE", "0")))
    res = run_bass_kernel_spmd(nc, in_maps, list(range(N_CORES)), trace=trace)
    LAST_RESULTS = res
    out = np.concatenate(
        [res.results[i]["out"].reshape(NB, C, OUT_L) for i in range(N_CORES)], axis=0
    )
    return np.ascontiguousarray(out.astype(np.float32))



# revision 2
# speedup vs baseline: 1.6368x; 1.6368x over previous
"""Dcls1d (dilated conv with learnable spacings, depthwise) Trainium2 kernel.

Problem: x [16, 256, 8192] f32, depthwise conv per channel with a 56-wide
kernel holding 7 interpolated taps (positions = k*8+4 + P, linear interp),
padding 27/27, plus bias.  Output [16, 256, 8191] f32.

Strategy (v2, "toeplitz"):
  - Channel-parallel: 32 channels x all 16 batches per NeuronCore.
  - Host pre-transposes the input to position-major layout: xt[p, t, f]
    holds xpad[l = t*128+p, c*16+b] in fp16, where xpad is the 27-left /
    zero-right padded input row.  Positions live on SBUF partitions, so
    the 56-tap convolution becomes a banded-Toeplitz matmul with the
    contraction over positions:
        out[u, (q,b)] = sum_m K[c][m, u] * xt[128q + m, (c,b)]
    split into piece A (m in [0,128), lhsT [128,128]) and piece B
    (m in [128,183), lhsT [55,128]) accumulated in PSUM.
  - No indirect gathers: per-channel tap positions are folded into the
    per-channel Toeplitz band K[c] on the host.  DMA traffic drops from
    ~67MB/core (7x gather re-read) to ~19MB/core (read once fp16, write
    once fp16 + 2.5MB weights).
  - L axis processed in 4 quarters (17-tile chunks with 1-tile halo
    overlap) so loads/compute/stores pipeline.  PSUM evacuation (+bias,
    fp32->fp16) alternates between the Scalar and Vector engines.
  - Output is stored transposed-blocked as out[c][u][q][b]; the host
    inverts the layout and casts to fp32.
"""

import os
from contextlib import ExitStack

import numpy as np

import concourse.bass as bass
import concourse.bacc as bacc_mod
import concourse.mybir as mybir
import concourse.tile as tile
from concourse.bass_utils import run_bass_kernel_spmd

# Problem geometry (hardcoded per spec nn_Dcls1d_12713103196284)
N, C, L = 16, 256, 8192
OUT_L = 8191
KS, DIL, PAD = 7, 8, 27
LK = DIL * KS  # 56
N_CORES = 8
CPC = C // N_CORES  # 32 channels per core
NB = N  # all batches on every core

# Position-major tiling
TP = 128  # positions per tile (partition dim)
NT = 65  # tiles: 65*128 = 8320 >= 27 + 8192 + 55
NQ = 64  # output blocks of 128 positions (64*128 = 8192 >= 8191)
QUARTERS = 4
QT = 16  # q-blocks per quarter
COLS = CPC * NB  # 512 columns = (channel, batch)
MB = 55  # piece-B contraction size (183 - 128)

F32 = mybir.dt.float32
F16 = mybir.dt.float16

_PROG = None
_PROG_IMPL = None
LAST_RESULTS = None  # test harness reads exec_time_ns off this


def _build_program_toeplitz():
    nc = bacc_mod.Bacc()
    xt = nc.dram_tensor("xt", [TP, NT, COLS], F16, kind="ExternalInput")
    wa = nc.dram_tensor("wa", [128, CPC * 128], F16, kind="ExternalInput")
    wb = nc.dram_tensor("wb", [MB, CPC * 128], F16, kind="ExternalInput")
    brep = nc.dram_tensor("brep", [128, CPC], F32, kind="ExternalInput")
    out = nc.dram_tensor("out", [CPC, 128, NQ * NB], F16, kind="ExternalOutput")

    with ExitStack() as ctx:
        tc = ctx.enter_context(tile.TileContext(nc))
        const = ctx.enter_context(tc.tile_pool(name="const", bufs=1))
        wa_sb = const.tile([128, CPC * 128], F16)
        nc.sync.dma_start(wa_sb[:], wa[:])
        wb_sb = const.tile([MB, CPC * 128], F16)
        nc.sync.dma_start(wb_sb[:], wb[:])
        brep_sb = const.tile([128, CPC], F32)
        nc.sync.dma_start(brep_sb[:], brep[:])

        xq_pool = ctx.enter_context(tc.tile_pool(name="xq", bufs=2))
        psum_pool = ctx.enter_context(tc.tile_pool(name="ps", bufs=8, space="PSUM"))
        st_pool = ctx.enter_context(tc.tile_pool(name="st", bufs=2))

        for Q in range(QUARTERS):
            t0 = Q * QT
            xq = xq_pool.tile([TP, QT + 1, COLS], F16, tag="xq", name="xq")
            nc.sync.dma_start(xq[:, :, :], xt[:, t0 : t0 + QT + 1, :])
            st = st_pool.tile([128, CPC, QT * NB], F16, tag="st", name="st")
            for c in range(CPC):
                ps = psum_pool.tile([128, QT, NB], F32, tag="ps", name="ps")
                nc.tensor.matmul(
                    out=ps[:, :, :],
                    lhsT=wa_sb[:, c * 128 : (c + 1) * 128],
                    rhs=xq[:, 0:QT, c * NB : (c + 1) * NB],
                    start=True,
                    stop=False,
                )
                nc.tensor.matmul(
                    out=ps[:, :, :],
                    lhsT=wb_sb[:, c * 128 : (c + 1) * 128],
                    rhs=xq[0:MB, 1 : QT + 1, c * NB : (c + 1) * NB],
                    start=False,
                    stop=True,
                )
                evs = st[:, c, :]
                if c % 2 == 0:
                    nc.scalar.activation(
                        evs,
                        ps[:, :, :],
                        mybir.ActivationFunctionType.Identity,
                        bias=brep_sb[:, c : c + 1],
                        scale=1.0,
                    )
                else:
                    nc.vector.tensor_scalar(
                        evs,
                        ps[:, :, :],
                        brep_sb[:, c : c + 1],
                        None,
                        mybir.AluOpType.add,
                    )
            nc.sync.dma_start(
                out[:, :, Q * QT * NB : (Q + 1) * QT * NB].rearrange(
                    "c p f -> p c f"
                ),
                st[:, :, :],
            )
    nc.finalize()
    return nc


def _host_kern56(weight, P):
    """Mirror reference.construct_kernel in float32: the dense 56-tap
    per-channel kernel (interp coefficients scattered at i0 / i0+1)."""
    w = np.asarray(weight, dtype=np.float32)[:, 0, :]  # [C, KS]
    Pm = np.asarray(P, dtype=np.float32)[0, :, 0, :]  # [C, KS]
    base = (np.arange(KS, dtype=np.float32) * DIL + DIL // 2).astype(np.float32)
    p = np.clip(Pm + base[None, :], np.float32(0.0), np.float32(LK - 1))
    i0f = np.floor(p)
    r = (p - i0f).astype(np.float32)
    i0 = i0f.astype(np.int32)
    i1 = np.minimum(i0 + 1, LK - 1)
    kern = np.zeros((C, LK), dtype=np.float32)
    rows = np.arange(C)[:, None].repeat(KS, axis=1)
    np.add.at(kern, (rows, i0), w * (np.float32(1.0) - r))
    np.add.at(kern, (rows, i1), w * r)
    return kern


def kernel(x, weight, P, bias):
    global _PROG, _PROG_IMPL, LAST_RESULTS
    impl = os.environ.get("KERNEL_IMPL", "toeplitz")
    x = np.asarray(x, dtype=np.float32)
    bias = np.asarray(bias, dtype=np.float32)
    kern = _host_kern56(weight, P)  # [C, 56] f32

    if _PROG is None or _PROG_IMPL != impl:
        _PROG = _build_program_toeplitz()
        _PROG_IMPL = impl
    nc = _PROG

    # Banded Toeplitz pieces per channel, fp16.
    m_idx = np.arange(128)[:, None]
    u_idx = np.arange(128)[None, :]
    la = m_idx - u_idx  # piece A tap index
    maska = (la >= 0) & (la < LK)
    lb = (np.arange(MB)[:, None] + 128) - u_idx  # piece B tap index
    maskb = (lb >= 0) & (lb < LK)
    kern16 = kern.astype(np.float16)

    # Host-transposed input, one shard per core: xt[p, t, c*16+b].
    # x [16, 256, 8192] -> [8 cores, 8192 L, 32 c, 16 b] fp16.
    xg = np.ascontiguousarray(
        x.reshape(N, N_CORES, CPC, L).transpose(1, 3, 2, 0)
    ).astype(np.float16)

    in_maps = []
    for core in range(N_CORES):
        ch = core * CPC + np.arange(CPC)
        ka = kern16[ch]  # [32, 56]
        A = np.zeros((CPC, 128, 128), dtype=np.float16)
        B = np.zeros((CPC, MB, 128), dtype=np.float16)
        A[:, maska] = ka[:, la[maska]]
        B[:, maskb] = ka[:, lb[maskb]]
        wa_arr = np.ascontiguousarray(A.transpose(1, 0, 2)).reshape(128, CPC * 128)
        wb_arr = np.ascontiguousarray(B.transpose(1, 0, 2)).reshape(MB, CPC * 128)
        brep_arr = np.broadcast_to(
            bias[ch].astype(np.float32)[None, :], (128, CPC)
        ).copy()

        xpadT = np.zeros((NT * TP, COLS), dtype=np.float16)
        xpadT[PAD : PAD + L] = xg[core].reshape(L, COLS)
        xt_arr = np.ascontiguousarray(
            xpadT.reshape(NT, TP, COLS).transpose(1, 0, 2)
        )
        in_maps.append(
            {"xt": xt_arr, "wa": wa_arr, "wb": wb_arr, "brep": brep_arr}
        )

    trace = bool(int(os.environ.get("KERNEL_TRACE", "0")))
    res = run_bass_kernel_spmd(nc, in_maps, list(range(N_CORES)), trace=trace)
    LAST_RESULTS = res

    out = np.empty((N, C, OUT_L), dtype=np.float32)
    for core in range(N_CORES):
        A = res.results[core]["out"].reshape(CPC, 128, NQ, NB)
        # out[b, ch[c], 128q+u] = A[c, u, q, b]
        full = A.transpose(3, 0, 2, 1).reshape(N, CPC, NQ * 128)
        out[:, core * CPC : (core + 1) * CPC, :] = full[:, :, :OUT_L]
    return np.ascontiguousarray(out)


# revision 6
# speedup vs baseline: 2.6472x; 1.6173x over previous
"""Dcls1d (dilated conv with learnable spacings, depthwise) Trainium2 kernel.

Problem: x [16, 256, 8192] f32, depthwise conv per channel with a 56-wide
kernel holding 7 interpolated taps (positions = k*8+4 + P, linear interp),
padding 27/27, plus bias.  Output [16, 256, 8191] f32.

Strategy ("toeplitz", v3):
  - Channel-parallel: 32 channels x all 16 batches per NeuronCore.
  - Host pre-transposes the input to position-major layout xt[p, c, t, b]
    (fp16): position l = t*128 + p of the 27-left zero-padded row of
    channel c, batch b.  Positions live on SBUF partitions, so the 56-tap
    depthwise conv becomes a banded-Toeplitz matmul contracting over
    positions:
        out[u, (q,b)] = sum_m K[c][m, u] * xpad[128q + m, (c,b)]
    piece A: m in [0,128), lhsT [128,128]; piece B: m in [128,183),
    lhsT [55,64] (only u >= 73 receive piece-B taps), accumulated in PSUM.
  - The (c-major) column layout makes every matmul rhs a single
    contiguous 256-element free run, which the PE streams at full rate
    (strided multi-dim rhs APs cost ~220ns/matmul in AP restarts).
  - No indirect gathers: per-channel tap positions are folded into the
    per-channel Toeplitz band K[c] on the host.  DMA traffic drops from
    ~67MB/core (7x gather re-read) to ~19MB/core.
  - L axis processed in 4 quarters (17-tile chunks, 1-tile halo) so
    loads/compute/stores pipeline.  PSUM evacuation (+bias, fp32->fp16)
    alternates between the Scalar and Vector engines.
  - Output is stored transposed-blocked as out[c][u][Q*256 + q*16 + b];
    the host inverts the layout and casts to fp32.
"""

import os
from contextlib import ExitStack

import numpy as np

import concourse.bass as bass
import concourse.bacc as bacc_mod
import concourse.mybir as mybir
import concourse.tile as tile
from concourse.bass_utils import run_bass_kernel_spmd

# Problem geometry (hardcoded per spec nn_Dcls1d_12713103196284)
N, C, L = 16, 256, 8192
OUT_L = 8191
KS, DIL, PAD = 7, 8, 27
LK = DIL * KS  # 56
N_CORES = 8
CPC = C // N_CORES  # 32 channels per core
NB = N  # all batches on every core

# Position-major tiling
TP = 128  # positions per tile (partition dim)
NT = 65  # tiles: 65*128 = 8320 >= 27 + 8192 + 55
NQ = 64  # output blocks of 128 positions
QUARTERS = 2
QT = 32  # q-blocks per chunk
MB = 128  # piece-B contraction padded to full 128 rows (55 real + 73 zero)
UB = 128  # piece-B output window (full; u < 73 rows get zero weights)
FQ = QT * NB  # 512 free columns per (channel, chunk)

F32 = mybir.dt.float32
F16 = mybir.dt.float16

_PROG = None
_PROG_IMPL = None
LAST_RESULTS = None  # test harness reads exec_time_ns off this


def _build_program_toeplitz():
    nc = bacc_mod.Bacc()
    xt = nc.dram_tensor("xt", [TP, CPC, NT * NB], F16, kind="ExternalInput")
    wa = nc.dram_tensor("wa", [128, CPC * 128], F16, kind="ExternalInput")
    wb = nc.dram_tensor("wb", [MB, CPC * UB], F16, kind="ExternalInput")
    brep = nc.dram_tensor("brep", [128, CPC], F32, kind="ExternalInput")
    out = nc.dram_tensor("out", [CPC, 128, NQ * NB], F16, kind="ExternalOutput")

    with ExitStack() as ctx:
        tc = ctx.enter_context(tile.TileContext(nc))
        const = ctx.enter_context(tc.tile_pool(name="const", bufs=1))
        wa_sb = const.tile([128, CPC * 128], F16)
        nc.sync.dma_start(wa_sb[:], wa[:])
        wb_sb = const.tile([MB, CPC * UB], F16)
        nc.sync.dma_start(wb_sb[:], wb[:])
        brep_sb = const.tile([128, CPC], F32)
        nc.sync.dma_start(brep_sb[:], brep[:])

        xq_pool = ctx.enter_context(tc.tile_pool(name="xq", bufs=2))
        psum_pool = ctx.enter_context(tc.tile_pool(name="ps", bufs=8, space="PSUM"))
        st_pool = ctx.enter_context(tc.tile_pool(name="st", bufs=2))

        CW = (QT + 1) * NB  # 272 columns per channel in a quarter chunk
        for Q in range(QUARTERS):
            xq = xq_pool.tile([TP, CPC, CW], F16, tag="xq", name="xq")
            nc.sync.dma_start(
                xq[:, :, :], xt[:, :, Q * QT * NB : Q * QT * NB + CW]
            )
            st = st_pool.tile([128, CPC, FQ], F16, tag="st", name="st")
            for c in range(CPC):
                ps = psum_pool.tile([128, FQ], F32, tag="ps", name="ps")
                nc.tensor.matmul(
                    out=ps[:, :],
                    lhsT=wa_sb[:, c * 128 : (c + 1) * 128],
                    rhs=xq[:, c, 0:FQ],
                    start=True,
                    stop=False,
                )
                nc.tensor.matmul(
                    out=ps[:, :],
                    lhsT=wb_sb[:, c * UB : (c + 1) * UB],
                    rhs=xq[:, c, NB : NB + FQ],
                    start=False,
                    stop=True,
                )
                evs = st[:, c, :]
                if c % 2 == 0:
                    nc.scalar.activation(
                        evs,
                        ps[:, :],
                        mybir.ActivationFunctionType.Identity,
                        bias=brep_sb[:, c : c + 1],
                        scale=1.0,
                    )
                else:
                    nc.vector.tensor_scalar(
                        evs,
                        ps[:, :],
                        brep_sb[:, c : c + 1],
                        None,
                        mybir.AluOpType.add,
                    )
            for c0 in range(0, CPC, 8):
                nc.sync.dma_start(
                    out[c0 : c0 + 8, :, Q * FQ : (Q + 1) * FQ].rearrange(
                        "c p f -> p c f"
                    ),
                    st[:, c0 : c0 + 8, :],
                )
    nc.finalize()
    return nc


def _host_kern56(weight, P):
    """Mirror reference.construct_kernel in float32: the dense 56-tap
    per-channel kernel (interp coefficients scattered at i0 / i0+1)."""
    w = np.asarray(weight, dtype=np.float32)[:, 0, :]  # [C, KS]
    Pm = np.asarray(P, dtype=np.float32)[0, :, 0, :]  # [C, KS]
    base = (np.arange(KS, dtype=np.float32) * DIL + DIL // 2).astype(np.float32)
    p = np.clip(Pm + base[None, :], np.float32(0.0), np.float32(LK - 1))
    i0f = np.floor(p)
    r = (p - i0f).astype(np.float32)
    i0 = i0f.astype(np.int32)
    i1 = np.minimum(i0 + 1, LK - 1)
    kern = np.zeros((C, LK), dtype=np.float32)
    rows = np.arange(C)[:, None].repeat(KS, axis=1)
    np.add.at(kern, (rows, i0), w * (np.float32(1.0) - r))
    np.add.at(kern, (rows, i1), w * r)
    return kern


def _host_inputs(x, weight, P, bias):
    kern16 = _host_kern56(weight, P).astype(np.float16)

    # Banded Toeplitz masks.
    m_idx = np.arange(128)[:, None]
    u_idx = np.arange(128)[None, :]
    la = m_idx - u_idx
    maska = (la >= 0) & (la < LK)
    ub_idx = np.arange(UB)[None, :]
    lb = (np.arange(55)[:, None] + 128) - ub_idx
    maskb = (lb >= 0) & (lb < LK)

    # x [16, 256, 8192] -> [8 cores, 8192 L, 32 c, 16 b] fp16
    xg = np.ascontiguousarray(
        x.reshape(N, N_CORES, CPC, L).transpose(1, 3, 2, 0)
    ).astype(np.float16)

    in_maps = []
    for core in range(N_CORES):
        ch = core * CPC + np.arange(CPC)
        ka = kern16[ch]  # [32, 56]
        A = np.zeros((CPC, 128, 128), dtype=np.float16)
        B = np.zeros((CPC, MB, UB), dtype=np.float16)
        Bv = np.zeros((CPC, 55, UB), dtype=np.float16)
        A[:, maska] = ka[:, la[maska]]
        Bv[:, maskb] = ka[:, lb[maskb]]
        B[:, 0:55, :] = Bv
        wa_arr = np.ascontiguousarray(A.transpose(1, 0, 2)).reshape(128, CPC * 128)
        wb_arr = np.ascontiguousarray(B.transpose(1, 0, 2)).reshape(MB, CPC * UB)
        brep_arr = np.broadcast_to(
            bias[ch].astype(np.float32)[None, :], (128, CPC)
        ).copy()

        # xt[p, c, t*16 + b] = xpad[t*128 + p, c, b]
        xpadT = np.zeros((NT * TP, CPC, NB), dtype=np.float16)
        xpadT[PAD : PAD + L] = xg[core]
        xt_arr = np.ascontiguousarray(
            xpadT.reshape(NT, TP, CPC, NB).transpose(1, 2, 0, 3)
        ).reshape(TP, CPC, NT * NB)
        in_maps.append(
            {"xt": xt_arr, "wa": wa_arr, "wb": wb_arr, "brep": brep_arr}
        )
    return in_maps


def kernel(x, weight, P, bias):
    global _PROG, _PROG_IMPL, LAST_RESULTS
    impl = os.environ.get("KERNEL_IMPL", "toeplitz")
    x = np.asarray(x, dtype=np.float32)
    bias = np.asarray(bias, dtype=np.float32)

    if _PROG is None or _PROG_IMPL != impl:
        _PROG = _build_program_toeplitz()
        _PROG_IMPL = impl
    nc = _PROG

    in_maps = _host_inputs(x, weight, P, bias)
    trace = bool(int(os.environ.get("KERNEL_TRACE", "0")))
    res = run_bass_kernel_spmd(nc, in_maps, list(range(N_CORES)), trace=trace)
    LAST_RESULTS = res

    out = np.empty((N, C, OUT_L), dtype=np.float32)
    for core in range(N_CORES):
        A = res.results[core]["out"].reshape(CPC, 128, NQ, NB)
        # out[b, ch[c], 128q+u] = A[c, u, q, b]
        full = A.transpose(3, 0, 2, 1).reshape(N, CPC, NQ * 128)
        out[:, core * CPC : (core + 1) * CPC, :] = full[:, :, :OUT_L]
    return np.ascontiguousarray(out)
